# revision 15
# baseline (speedup 1.0000x reference)
"""FFF (fast feedforward / MoE-routing binary tree) forward pass on 8 Trainium2 NeuronCores.

Strategy (data-parallel over the 16384-token batch, 2048 tokens/core):
  - Levels 0..7 (255 nodes) are computed DENSE: logits via PE fp32 matmul,
    tree walk via one-hot map maintenance on DVE, masked acts @ w_out.T via PE.
  - Levels 8..11 (3840 nodes) are computed SPARSE: each token only needs one
    node per level, so we gather w_in rows by the walked node index
    (indirect DMA), form the logit with a fused multiply-reduce on DVE, and
    accumulate coef * w_outT[idx] into the same PSUM banks via a diagonal
    fp32 matmul on PE.
  Host pre-transposes x tiles / shallow weights so no on-device transposes of
  inputs are needed (PE only transposes the 255-wide masked activations).
"""

import numpy as np

P = 128
D = 1024
KC = 8                 # 1024 / 128 contraction chunks
N_NODES = 4095
SH_NODES = 255         # nodes in levels 0..7
SHN = 256              # padded
DEPTH = 11
N_CORES = 8
TOK = 2048             # tokens per core
NT = TOK // P          # 16 token tiles per core


def build_nc():
    import os
    from concourse import bacc, bass, mybir, tile
    from concourse.masks import make_identity

    stage = os.environ.get("KERNEL_STAGE", "full")
    deep_on = stage not in ("shallow",)
    batch_gather = stage in ("batchgather",)  # (128,4)-idx gather is broken on HW
    deep_mm_on = stage not in ("nodeepmm",)
    debug_dump = os.environ.get("KERNEL_DEBUG", "0") == "1"

    dt = mybir.dt
    AFT = mybir.ActivationFunctionType
    ALU = mybir.AluOpType

    nc = bacc.Bacc("TRN2", target_bir_lowering=False, debug=False)

    x_d = nc.dram_tensor("x", [TOK, D], dt.float32, kind="ExternalInput")
    xT_d = nc.dram_tensor("xT", [NT, KC, P, P], dt.float32, kind="ExternalInput")
    w_in_d = nc.dram_tensor("w_in", [N_NODES, D], dt.float32, kind="ExternalInput")
    w_inT_sh_d = nc.dram_tensor("w_inT_sh", [KC, P, SHN], dt.float32, kind="ExternalInput")
    woT_sh_d = nc.dram_tensor("woT_sh", [2, P, D], dt.float32, kind="ExternalInput")
    w_outT_d = nc.dram_tensor("w_outT", [N_NODES, D], dt.float32, kind="ExternalInput")
    out_d = nc.dram_tensor("out", [TOK, D], dt.float32, kind="ExternalOutput")
    dbg = {}

    if debug_dump:
        dbg["logits"] = nc.dram_tensor("dbg_logits", [NT, P, SHN], dt.float32, kind="ExternalOutput")
        dbg["map"] = nc.dram_tensor("dbg_map", [NT, P, SHN], dt.float32, kind="ExternalOutput")
        dbg["mskT"] = nc.dram_tensor("dbg_mskT", [NT, P, 2 * P], dt.float32, kind="ExternalOutput")
        dbg["idx"] = nc.dram_tensor("dbg_idx", [NT, P, 4], dt.int32, kind="ExternalOutput")
        dbg["coef"] = nc.dram_tensor("dbg_coef", [NT, P, 4], dt.float32, kind="ExternalOutput")
        dbg["wog"] = nc.dram_tensor("dbg_wog", [NT, P, 4 * D], dt.float32, kind="ExternalOutput")

    with tile.TileContext(nc) as tc:
        with (
            tc.tile_pool(name="const", bufs=1) as cpool,
            tc.tile_pool(name="xT", bufs=3) as xT_pool,
            tc.tile_pool(name="xn", bufs=4) as xn_pool,
            tc.tile_pool(name="small", bufs=3) as small_pool,
            tc.tile_pool(name="tiny", bufs=4) as tiny_pool,
            tc.tile_pool(name="mskT", bufs=3) as mskT_pool,
            tc.tile_pool(name="win", bufs=4) as win_pool,
            tc.tile_pool(name="wout", bufs=3) as wout_pool,
            tc.tile_pool(name="dscr", bufs=2) as dscr_pool,
            tc.tile_pool(name="osb", bufs=3) as osb_pool,
            tc.tile_pool(name="lps", bufs=2, space="PSUM") as lps_pool,
            tc.tile_pool(name="tps", bufs=2, space="PSUM") as tps_pool,
            tc.tile_pool(name="ops", bufs=2, space="PSUM") as ops_pool,
        ):
            ident = cpool.tile([P, P], dt.float32)
            make_identity(nc, ident[:])
            w_inT_sb = cpool.tile([P, KC * SHN], dt.float32)
            nc.sync.dma_start(
                out=w_inT_sb[:].rearrange("p (k n) -> p k n", k=KC),
                in_=w_inT_sh_d[:].rearrange("k p n -> p k n"),
            )
            woT_sb = cpool.tile([P, 2 * D], dt.float32)
            nc.sync.dma_start(
                out=woT_sb[:].rearrange("p (c o) -> p c o", c=2),
                in_=woT_sh_d[:].rearrange("c p o -> p c o"),
            )

            for t in range(NT):
                xT = xT_pool.tile([P, D], dt.float32)
                nc.sync.dma_start(
                    out=xT[:].rearrange("p (k j) -> p k j", k=KC),
                    in_=xT_d[t].rearrange("k p j -> p k j"),
                )
                xn = xn_pool.tile([P, D], dt.float32)
                nc.sync.dma_start(out=xn[:], in_=x_d[t * P:(t + 1) * P, :])

                # ---- dense shallow logits: (128 tokens, 256 nodes) ----
                lps = lps_pool.tile([P, SHN], dt.float32, space="PSUM")
                for k in range(KC):
                    nc.tensor.matmul(
                        out=lps[:],
                        lhsT=xT[:, k * P:(k + 1) * P],
                        rhs=w_inT_sb[:, k * SHN:(k + 1) * SHN],
                        start=(k == 0),
                        stop=(k == KC - 1),
                    )
                lsb = small_pool.tile([P, SHN], dt.float32, tag="lsb")
                nc.scalar.copy(out=lsb[:], in_=lps[:])
                acts = small_pool.tile([P, SHN], dt.float32, tag="acts")
                nc.scalar.activation(out=acts[:], in_=lps[:], func=AFT.Gelu)

                # ---- shallow walk: one-hot decision map + heap index r ----
                mp = small_pool.tile([P, SHN], dt.float32, tag="map")
                nc.vector.memset(mp[:], 0.0)
                nc.vector.memset(mp[:, 0:1], 1.0)
                r = tiny_pool.tile([P, 1], dt.float32, tag="r")
                s2 = tiny_pool.tile([P, 1], dt.float32, tag="s2")
                pick = tiny_pool.tile([P, 1], dt.float32, tag="pick")
                dec = tiny_pool.tile([P, P], dt.float32, tag="dec")
                scr = tiny_pool.tile([P, P], dt.float32, tag="scr")
                # level 0: map[1]=1-dec0, map[2]=dec0, r=2+dec0
                nc.vector.tensor_scalar(
                    out=mp[:, 2:3], in0=lsb[:, 0:1], scalar1=0.0, scalar2=None, op0=ALU.is_gt
                )
                nc.vector.tensor_scalar(
                    out=mp[:, 1:2], in0=lsb[:, 0:1], scalar1=0.0, scalar2=None, op0=ALU.is_le
                )
                nc.vector.tensor_scalar(
                    out=r[:], in0=mp[:, 2:3], scalar1=2.0, scalar2=None, op0=ALU.add
                )
                for d in range(1, 8):
                    o = 2 ** d - 1
                    w = 2 ** d
                    nc.vector.tensor_scalar(
                        out=dec[:, :w], in0=lsb[:, o:o + w],
                        scalar1=0.0, scalar2=None, op0=ALU.is_gt,
                    )
                    nc.vector.tensor_scalar(
                        out=s2[:], in0=r[:], scalar1=2.0, scalar2=None, op0=ALU.mult
                    )
                    if d < 7:
                        o1 = 2 ** (d + 1) - 1
                        nxt = mp[:, o1:o1 + 2 * w].rearrange("p (n two) -> p n two", two=2)
                        # odd slots = OH*dec
                        nc.vector.tensor_tensor(
                            out=nxt[:, :, 1], in0=mp[:, o:o + w], in1=dec[:, :w],
                            op=ALU.mult,
                        )
                        nc.vector.tensor_reduce(
                            out=pick[:], in_=nxt[:, :, 1],
                            axis=mybir.AxisListType.X, op=ALU.add,
                        )
                        # even slots = OH - odd
                        nc.vector.tensor_tensor(
                            out=nxt[:, :, 0], in0=mp[:, o:o + w], in1=nxt[:, :, 1],
                            op=ALU.subtract,
                        )
                    else:
                        nc.vector.tensor_tensor(
                            out=scr[:, :w], in0=mp[:, o:o + w], in1=dec[:, :w],
                            op=ALU.mult,
                        )
                        nc.vector.tensor_reduce(
                            out=pick[:], in_=scr[:, :w],
                            axis=mybir.AxisListType.X, op=ALU.add,
                        )
                    nc.vector.tensor_tensor(out=r[:], in0=s2[:], in1=pick[:], op=ALU.add)

                # ---- masked acts + transpose for mm2 ----
                msk = small_pool.tile([P, SHN], dt.float32, tag="msk")
                nc.vector.tensor_tensor(out=msk[:], in0=acts[:], in1=mp[:], op=ALU.mult)
                mskT = mskT_pool.tile([P, 2 * P], dt.float32)
                for c in range(2):
                    tp = tps_pool.tile([P, P], dt.float32, space="PSUM")
                    nc.tensor.transpose(
                        out=tp[:], in_=msk[:, c * P:(c + 1) * P], identity=ident[:]
                    )
                    nc.scalar.copy(out=mskT[:, c * P:(c + 1) * P], in_=tp[:])

                # ---- deep levels 8..11: gather + fused dot + walk ----
                coef4 = tiny_pool.tile([P, 4], dt.float32, tag="coef4")
                idx4 = tiny_pool.tile([P, 4], dt.int32, tag="idx4")
                idxf = tiny_pool.tile([P, 1], dt.float32, tag="idxf")
                logit = tiny_pool.tile([P, 1], dt.float32, tag="logit")
                dscr = dscr_pool.tile([P, D], dt.float32)
                for l in range(4 if deep_on else 0):
                    dlev = 8 + l
                    nc.vector.tensor_scalar(
                        out=idxf[:], in0=r[:], scalar1=-1.0, scalar2=None, op0=ALU.add
                    )
                    nc.vector.tensor_copy(out=idx4[:, l:l + 1], in_=idxf[:])
                    gw = win_pool.tile([P, D], dt.float32)
                    nc.gpsimd.indirect_dma_start(
                        out=gw[:],
                        out_offset=None,
                        in_=w_in_d[:],
                        in_offset=bass.IndirectOffsetOnAxis(ap=idx4[:, l:l + 1], axis=0),
                    )
                    nc.vector.tensor_tensor(
                        out=dscr[:], in0=xn[:], in1=gw[:], op=ALU.mult
                    )
                    nc.vector.tensor_reduce(
                        out=logit[:], in_=dscr[:],
                        axis=mybir.AxisListType.X, op=ALU.add,
                    )
                    nc.scalar.activation(out=coef4[:, l:l + 1], in_=logit[:], func=AFT.Gelu)
                    if dlev < DEPTH:
                        nc.vector.tensor_scalar(
                            out=dec[:, 0:1], in0=logit[:], scalar1=0.0, scalar2=None,
                            op0=ALU.is_gt,
                        )
                        nc.vector.tensor_scalar(
                            out=s2[:], in0=r[:], scalar1=2.0, scalar2=None, op0=ALU.mult
                        )
                        nc.vector.tensor_tensor(
                            out=r[:], in0=s2[:], in1=dec[:, 0:1], op=ALU.add
                        )

                # batched gather of the 4 w_outT rows per token
                wog = wout_pool.tile([P, 4 * D], dt.float32)
                if deep_on:
                    if batch_gather:
                        nc.gpsimd.indirect_dma_start(
                            out=wog[:],
                            out_offset=None,
                            in_=w_outT_d[:],
                            in_offset=bass.IndirectOffsetOnAxis(ap=idx4[:], axis=0),
                        )
                    else:
                        for l in range(4):
                            nc.gpsimd.indirect_dma_start(
                                out=wog[:, l * D:(l + 1) * D],
                                out_offset=None,
                                in_=w_outT_d[:],
                                in_offset=bass.IndirectOffsetOnAxis(
                                    ap=idx4[:, l:l + 1], axis=0
                                ),
                            )

                # ---- output accumulation in PSUM ----
                ops = ops_pool.tile([P, D], dt.float32, space="PSUM")
                # shallow: maskedT.T @ w_outT[0:255]  (2 K-chunks x 2 N-halves)
                dmm = deep_on and deep_mm_on
                for h in range(2):
                    for c in range(2):
                        nc.tensor.matmul(
                            out=ops[:, h * 512:(h + 1) * 512],
                            lhsT=mskT[:, c * P:(c + 1) * P],
                            rhs=woT_sb[:, c * D + h * 512: c * D + h * 512 + 512],
                            start=(c == 0),
                            stop=(c == 1 and not dmm),
                            skip_group_check=True,
                        )
                # deep: diag(coef_l) @ gathered w_outT rows
                for l in range(4 if dmm else 0):
                    dg = tiny_pool.tile([P, P], dt.float32, tag="diag")
                    nc.vector.tensor_scalar(
                        out=dg[:], in0=ident[:], scalar1=coef4[:, l:l + 1],
                        scalar2=None, op0=ALU.mult,
                    )
                    for h in range(2):
                        nc.tensor.matmul(
                            out=ops[:, h * 512:(h + 1) * 512],
                            lhsT=dg[:],
                            rhs=wog[:, l * D + h * 512: l * D + h * 512 + 512],
                            start=False,
                            stop=(l == 3),
                            skip_group_check=True,
                        )

                osb = osb_pool.tile([P, D], dt.float32)
                nc.scalar.copy(out=osb[:], in_=ops[:])
                nc.sync.dma_start(out=out_d[t * P:(t + 1) * P, :], in_=osb[:])

                if debug_dump:
                    nc.sync.dma_start(out=dbg["logits"][t], in_=lsb[:])
                    nc.sync.dma_start(out=dbg["map"][t], in_=mp[:])
                    nc.sync.dma_start(out=dbg["mskT"][t], in_=mskT[:])
                    nc.sync.dma_start(out=dbg["idx"][t], in_=idx4[:])
                    nc.sync.dma_start(out=dbg["coef"][t], in_=coef4[:])
                    nc.sync.dma_start(out=dbg["wog"][t], in_=wog[:])

    nc.compile()
    return nc


def host_prep(x, w_in, w_out):
    """Build the per-core input maps (host-side transposes/tilings)."""
    x = np.ascontiguousarray(x, np.float32)
    w_in = np.ascontiguousarray(w_in, np.float32)
    w_out = np.ascontiguousarray(w_out, np.float32)

    w_inT_sh = np.zeros((SHN, D), np.float32)
    w_inT_sh[:SH_NODES] = w_in[:SH_NODES]
    w_inT_sh = np.ascontiguousarray(
        w_inT_sh.T.reshape(KC, P, SHN)
    )  # [k,p,n] = w_in[n, k*128+p]

    woT_sh = np.zeros((SHN, D), np.float32)
    woT_sh[:SH_NODES] = w_out[:, :SH_NODES].T
    woT_sh = np.ascontiguousarray(woT_sh.reshape(2, P, D))  # [c,p,o] = w_out[o, c*128+p]

    w_outT = np.ascontiguousarray(w_out.T)  # (4095, 1024)

    in_maps = []
    for c in range(N_CORES):
        xs = x[c * TOK:(c + 1) * TOK]
        xT = np.ascontiguousarray(
            xs.reshape(NT, P, KC, P).transpose(0, 2, 3, 1)
        )  # [t,k,p,j] = xs[t*128+j, k*128+p]
        in_maps.append(
            {
                "x": np.ascontiguousarray(xs),
                "xT": xT,
                "w_in": w_in,
                "w_inT_sh": w_inT_sh,
                "woT_sh": woT_sh,
                "w_outT": w_outT,
            }
        )
    return in_maps


_NC_CACHE = {}


def kernel(x, w_in, w_out, force_depth=None, **_ignored):
    from concourse.bass_utils import run_bass_kernel_spmd

    if "nc" not in _NC_CACHE:
        _NC_CACHE["nc"] = build_nc()
    nc = _NC_CACHE["nc"]

    in_maps = host_prep(np.asarray(x), np.asarray(w_in), np.asarray(w_out))
    res = run_bass_kernel_spmd(nc, in_maps, core_ids=list(range(N_CORES)))
    out = np.concatenate([res.results[c]["out"] for c in range(N_CORES)], axis=0)
    return out.astype(np.float32)


if __name__ == "__main__":
    import reference

    inputs = reference.setup_inputs()
    expected = np.asarray(reference.reference(**inputs))
    actual = kernel(**{k: np.asarray(v) for k, v in inputs.items()})
    err = np.abs(actual - expected).max()
    print("absmax err:", err)
